# revision 1
# baseline (speedup 1.0000x reference)
"""Trainium2 Bass kernel for nn_NegativeSoftmax (few-shot episode adaptation).

Math: the adapted weight W_t stays in span{W0 rows, pooled support features}:
    W_t[c,:] = a_t * W0[c,:] + sum_s B_t[c,s] * sxp[s,:]
where a_t is a data-independent scalar sequence (weight decay on the W0
component) and B_t [5,25] evolves via the SGD+momentum recurrence driven by
per-step softmax gradients.  The z.min subtraction in the reference has zero
gradient (softmax-gradient rows sum to 0) and doesn't move argmax, so it is
dropped.  The 700 sequential steps then only need [25]-sized linear algebra.

Per step (all on-chip, Bt layout [s,c], B-history absorbs the momentum state):
    Y10_t = KB.T @ btB_t + KA.T @ btA_t          # 10*(K@B + a_t*z0 - 0.4*OH)
    P = exp(Y10 - rowmax); ssum = rowsum(P)      # one ACT op (accum_out)
    rw = wcol_t / ssum
    B_{t+1} = P*rw + [(1+MOM+cwd)*B_t - MOM*B_{t-1} + OHWn_t]
with K = sxp@sxp.T, z0 = sxp@W0.T computed on device.  Final scoring:
    scores ~ qs.T@B + 25*q0.T@(a*I)  -> argmax -> compare labels.

Distribution: the sequential loop is replicated on all 8 cores; each core
pools + scores its own 1/8 slice of the 600 queries (the only big data).
Outputs are int32 rewards gathered on host.
"""

import numpy as np

SCALE, MARGIN, LR, MOM, DAMP, WD = 10.0, 0.4, 1.01, 0.9, 0.9, 1e-3
N_CORES = 8

_CACHE = {}


def _host_a_seq(T):
    a, va = np.float32(1.0), np.float32(0.0)
    seq = [np.float32(a)]
    for t in range(T):
        d = np.float32(WD) * a
        va = d if t == 0 else np.float32(MOM) * va + np.float32(1.0 - DAMP) * d
        a = a - np.float32(LR) * va
        seq.append(np.float32(a))
    return np.asarray(seq, np.float32)


def _build_program(T, QL, n_cls, S, C, qred_chunk=5, act_accum=True):
    import concourse.bacc as bacc
    import concourse.mybir as mybir
    import concourse.tile as tile

    f32 = mybir.dt.float32
    i32 = mybir.dt.int32
    NT = C // 128   # channel tiles
    NA = 2 * n_cls  # aug rows (a_t*I ; I)
    NB = n_cls      # block width

    nc = bacc.Bacc("TRN2", target_bir_lowering=False, name="negsoftmax")
    d_qx = nc.dram_tensor("qx", [C, QL, 25], f32, kind="ExternalInput")
    d_sx = nc.dram_tensor("sx", [C, S, 25], f32, kind="ExternalInput")
    d_w0t = nc.dram_tensor("w0t", [C, n_cls], f32, kind="ExternalInput")
    d_oht4 = nc.dram_tensor("oht4", [n_cls, S], f32, kind="ExternalInput")
    # btA rows for every step t=0..T: [a_t*I ; I] blocks
    d_augr = nc.dram_tensor("augr", [NA, NB * (T + 1)], f32, kind="ExternalInput")
    d_wcol = nc.dram_tensor("wcol", [S, T], f32, kind="ExternalInput")
    d_ohwn = nc.dram_tensor("ohwn", [S, NB * T], f32, kind="ExternalInput")
    d_ycmp = nc.dram_tensor("ycmp", [QL, 1], f32, kind="ExternalInput")
    d_desc = nc.dram_tensor("desc", [QL, n_cls], f32, kind="ExternalInput")
    d_rew = nc.dram_tensor("rew", [QL, 1], i32, kind="ExternalOutput")

    AX = mybir.AxisListType.X
    OP = mybir.AluOpType
    EXP = mybir.ActivationFunctionType.Exp

    with tile.TileContext(nc) as tc:
        with (
            tc.tile_pool(name="persist", bufs=1) as pp,
            tc.tile_pool(name="sraw", bufs=6) as sraw_pool,
            tc.tile_pool(name="step", bufs=4) as sp,
            tc.tile_pool(name="psum", bufs=2, space="PSUM") as psp,
            tc.tile_pool(name="psum_keep", bufs=1, space="PSUM") as pkp,
        ):
            # ---------------- persistent tiles ----------------
            # loop matmul lhsT is split at the 32-partition alignment rule:
            #   kb [25,25] = 10/625*Kraw   (B rows)
            #   ka [10,25] = [10/25*z0T ; -4*OH.T]  (aug rows)
            kb = pp.tile([S, S], f32)
            ka = pp.tile([NA, S], f32)
            # btB[:, NB*t:NB*(t+1)] = B_t ; btA likewise holds [a_t*I ; I]
            btb = pp.tile([S, NB * (T + 1)], f32)
            bta = pp.tile([NA, NB * (T + 1)], f32)
            wcol_sb = pp.tile([S, T], f32)
            ohwn_sb = pp.tile([S, NB * T], f32)
            ycmp_sb = pp.tile([QL, 1], f32)
            desc_sb = pp.tile([QL, n_cls], f32)
            sxsum = pp.tile([128, NT * S], f32)    # pooled (sum) support, c-major
            w0t_sb = pp.tile([128, NT * n_cls], f32)
            qxraw = pp.tile([128, NT * QL * 25], f32)  # raw query features
            qxsum = pp.tile([128, NT * QL], f32)
            sq = pp.tile([S, QL], f32)             # qsrawT   (scoring lhsT, B rows)
            sq0 = pp.tile([n_cls, QL], f32)        # 25*q0rawT (scoring lhsT, a rows)

            # ---------------- support pooling + K/z0 ----------------
            # one DMA for all of w0t (straight into its SBUF layout), then
            # the sx tiles, alternating the two HWDGE rings (SP / Activation)
            nc.scalar.dma_start(
                w0t_sb[:].rearrange("p (j c) -> p j c", j=NT),
                d_w0t[:].rearrange("(j p) c -> p j c", p=128))
            for j in range(NT):
                eng = nc.sync if j % 2 == 0 else nc.scalar
                sxr = sraw_pool.tile([128, S, 25], f32, tag="sxr")
                eng.dma_start(sxr[:], d_sx[128 * j:128 * (j + 1)])
                nc.vector.tensor_reduce(
                    out=sxsum[:, S * j:S * (j + 1)], in_=sxr[:], axis=AX, op=OP.add)

            # small loop-constant DMAs (needed only by loop start ~20us in)
            nc.sync.dma_start(wcol_sb[:], d_wcol[:])
            nc.sync.dma_start(ohwn_sb[:], d_ohwn[:])
            nc.sync.dma_start(bta[:], d_augr[:])
            nc.sync.dma_start(ka[n_cls:NA, :], d_oht4[:])
            nc.sync.dma_start(ycmp_sb[:], d_ycmp[:])
            nc.sync.dma_start(desc_sb[:], d_desc[:])
            nc.vector.memset(btb[:, 0:NB], 0.0)

            kraw = pkp.tile([S, S], f32, tag="kraw")
            z0t = pkp.tile([n_cls, S], f32, tag="z0t")
            for j in range(NT):
                nc.tensor.matmul(
                    kraw[:], sxsum[:, S * j:S * (j + 1)], sxsum[:, S * j:S * (j + 1)],
                    start=(j == 0), stop=(j == NT - 1))
            for j in range(NT):
                nc.tensor.matmul(
                    z0t[:], w0t_sb[:, n_cls * j:n_cls * (j + 1)],
                    sxsum[:, S * j:S * (j + 1)],
                    start=(j == 0), stop=(j == NT - 1))
            kb_copy = nc.scalar.mul(kb[:], kraw[:], 10.0 / 625.0)
            nc.scalar.mul(ka[0:n_cls, :], z0t[:], 10.0 / 25.0)

            # query raw loads: gate on kb so they don't steal DMA bandwidth
            # from the prologue (SP runs ahead and would start them at t=0)
            from concourse.tile import add_dep_helper
            for j in range(NT):
                dq = nc.sync.dma_start(
                    qxraw[:, QL * 25 * j:QL * 25 * (j + 1)],
                    d_qx[128 * j:128 * (j + 1)])
                add_dep_helper(dq.ins, kb_copy.ins, sync=True,
                               reason="defer qx DMA past prologue")

            # ---------------- the T-step adaptation loop ----------------
            for t in range(T):
                cwd = -LR * (1.0 if t == 0 else 1.0 - DAMP) * WD
                bt_cur = btb[:, NB * t:NB * (t + 1)]
                tp = max(t - 1, 0)
                bt_prev = btb[:, NB * tp:NB * tp + NB]
                bt_next = btb[:, NB * (t + 1):NB * (t + 2)]

                y10 = psp.tile([S, n_cls], f32, tag="y10")
                nc.tensor.matmul(y10[:], kb[:], bt_cur, start=True, stop=False)
                nc.tensor.matmul(y10[:], ka[:], bta[:, NB * t:NB * (t + 1)],
                                 start=False, stop=True)

                nmax = sp.tile([S, 1], f32, tag="nmax")
                nc.vector.tensor_reduce(
                    out=nmax[:], in_=y10[:], axis=AX, op=OP.max, negate=True)
                p = sp.tile([S, n_cls], f32, tag="p")
                ssum = sp.tile([S, 1], f32, tag="ssum")
                if act_accum:
                    nc.scalar.activation(p[:], y10[:], EXP,
                                         bias=nmax[:, 0:1], scale=1.0,
                                         accum_out=ssum[:])
                else:
                    nc.scalar.activation(p[:], y10[:], EXP,
                                         bias=nmax[:, 0:1], scale=1.0)
                    nc.vector.tensor_reduce(
                        out=ssum[:], in_=p[:], axis=AX, op=OP.add)

                # off-chain (Pool): g2 = (1+MOM+cwd)*B_t - MOM*B_{t-1} + OHWn_t
                g1a = sp.tile([S, n_cls], f32, tag="g1a")
                g1 = sp.tile([S, n_cls], f32, tag="g1")
                g2a = sp.tile([S, n_cls], f32, tag="g2a")
                g2 = sp.tile([S, n_cls], f32, tag="g2")
                nc.gpsimd.tensor_scalar_mul(g1a[:], bt_prev, -MOM)
                nc.gpsimd.tensor_add(g1[:], g1a[:], ohwn_sb[:, NB * t:NB * (t + 1)])
                nc.gpsimd.tensor_scalar_mul(g2a[:], bt_cur, 1.0 + MOM + cwd)
                nc.gpsimd.tensor_add(g2[:], g2a[:], g1[:])

                # in-chain DVE tail
                rs = sp.tile([S, 1], f32, tag="rs")
                rw = sp.tile([S, 1], f32, tag="rw")
                nc.vector.reciprocal(rs[:], ssum[:])
                nc.vector.tensor_mul(rw[:], rs[:], wcol_sb[:, t:t + 1])
                nc.vector.scalar_tensor_tensor(
                    out=bt_next, in0=p[:], scalar=rw[:, 0:1], in1=g2[:],
                    op0=OP.mult, op1=OP.add)

            # ------------- query pooling (gap-fills the loop) -------
            qv = qxraw[:].rearrange("p (j q s) -> p j q s", j=NT, q=QL)
            for j in range(NT):
                for q0 in range(0, QL, qred_chunk):
                    q1 = min(q0 + qred_chunk, QL)
                    nc.vector.tensor_reduce(
                        out=qxsum[:, QL * j + q0:QL * j + q1],
                        in_=qv[:, j, q0:q1], axis=AX, op=OP.add)
            qst = pkp.tile([S, QL], f32, tag="qst")
            q0t = pkp.tile([n_cls, QL], f32, tag="q0t")
            for j in range(NT):
                nc.tensor.matmul(
                    qst[:], sxsum[:, S * j:S * (j + 1)], qxsum[:, QL * j:QL * (j + 1)],
                    start=(j == 0), stop=(j == NT - 1))
            for j in range(NT):
                nc.tensor.matmul(
                    q0t[:], w0t_sb[:, n_cls * j:n_cls * (j + 1)],
                    qxsum[:, QL * j:QL * (j + 1)],
                    start=(j == 0), stop=(j == NT - 1))
            nc.scalar.mul(sq[:], qst[:], 1.0)
            nc.scalar.mul(sq0[:], q0t[:], 25.0)

            scores = pkp.tile([QL, n_cls], f32, tag="scores")
            nc.tensor.matmul(scores[:], sq[:], btb[:, NB * T:NB * (T + 1)],
                             start=True, stop=False)
            nc.tensor.matmul(scores[:], sq0[:], bta[0:n_cls, NB * T:NB * (T + 1)],
                             start=False, stop=True)

            mx = pp.tile([QL, 1], f32)
            eq = pp.tile([QL, n_cls], f32)
            vv = pp.tile([QL, n_cls], f32)
            rr = pp.tile([QL, 1], f32)
            okf = pp.tile([QL, 1], f32)
            oki = pp.tile([QL, 1], i32)
            nc.vector.tensor_reduce(out=mx[:], in_=scores[:], axis=AX, op=OP.max)
            nc.vector.tensor_scalar(
                out=eq[:], in0=scores[:], scalar1=mx[:, 0:1], scalar2=None,
                op0=OP.is_equal)
            nc.vector.tensor_mul(vv[:], eq[:], desc_sb[:])
            nc.vector.tensor_reduce(out=rr[:], in_=vv[:], axis=AX, op=OP.max)
            nc.vector.tensor_scalar(
                out=okf[:], in0=rr[:], scalar1=ycmp_sb[:, 0:1], scalar2=None,
                op0=OP.is_equal)
            nc.vector.tensor_copy(oki[:], okf[:])
            nc.sync.dma_start(d_rew[:], oki[:])

    nc.compile()
    return nc


def kernel(support_xf, support_y, query_xf, query_y, n_way, k_shot,
           batch_ids, batch_mask, weight_init, **_unused):
    import os
    # the axon NTFF-trace hook module isn't shipped in this container;
    # make sure a stray BASS_TRACE can't route us into that import
    os.environ["BASS_NEVER_TRACE"] = "1"
    from concourse.bass_utils import run_bass_kernel_spmd

    f32 = np.float32
    support_xf = np.ascontiguousarray(np.asarray(support_xf, f32))
    query_xf = np.ascontiguousarray(np.asarray(query_xf, f32))
    W0 = np.asarray(weight_init, f32)
    sy = np.asarray(support_y).reshape(-1).astype(np.int64)
    qy = np.asarray(query_y).reshape(-1).astype(np.int64)
    ids = np.asarray(batch_ids)
    mk = np.asarray(batch_mask)

    n_cls = W0.shape[0]
    S = support_xf.shape[1]
    C = support_xf.shape[2]
    T = ids.shape[0]
    Q = query_xf.shape[1]
    QL = (Q + N_CORES - 1) // N_CORES

    # ---- host preprocessing (layout + index/mask-derived constants) ----
    sx_cm = support_xf.reshape(S, C, 25).transpose(1, 0, 2).copy()   # [C,S,25]
    qx_cm = query_xf.reshape(Q, C, 25).transpose(1, 0, 2)            # [C,Q,25]
    if QL * N_CORES != Q:
        pad = QL * N_CORES - Q
        qx_cm = np.concatenate([qx_cm, np.zeros((C, pad, 25), f32)], axis=1)
        qy = np.concatenate([qy, np.zeros(pad, np.int64)])

    a_seq = _host_a_seq(T)
    I5 = np.eye(n_cls, dtype=f32)
    # augr rows: [a_t*I ; I] for t = 0..T, laid out as [2*n_cls, n_cls*(T+1)]
    augr = np.empty((T + 1, 2 * n_cls, n_cls), f32)
    augr[:, :n_cls, :] = a_seq[:, None, None] * I5[None]
    augr[:, n_cls:, :] = I5[None]
    augr_flat = augr.transpose(1, 0, 2).reshape(2 * n_cls, n_cls * (T + 1)).copy()

    m = mk.astype(f32)
    cnt = m.sum(1)
    w0w = np.zeros((T, S), f32)
    for b in range(ids.shape[1]):
        np.add.at(w0w, (np.arange(T), ids[:, b]), m[:, b])
    w0w /= cnt[:, None]
    kk = np.full(T, 1.0 - DAMP, f32)
    kk[0] = 1.0
    wcol = (-LR * kk[:, None] * SCALE * w0w).astype(f32)             # [T,S]
    OH = I5[sy]                                                      # [S,5]
    ohwn = (-wcol[:, :, None] * OH[None]).astype(f32)                # [T,S,5]
    ohwn_flat = ohwn.transpose(1, 0, 2).reshape(S, n_cls * T).copy()
    wcol_flat = wcol.T.copy()                                        # [S,T]
    oht4 = (-4.0 * OH.T).copy()                                      # [5,S]
    w0t = W0.T.copy()                                                # [C,5]
    desc = np.broadcast_to(
        np.arange(n_cls, 0, -1, dtype=f32)[None, :], (QL, n_cls)).copy()
    ycmp_all = (f32(n_cls) - qy.astype(f32)).reshape(N_CORES, QL, 1)

    key = (T, QL, n_cls, S, C)
    if key not in _CACHE:
        _CACHE[key] = _build_program(T, QL, n_cls, S, C)
    nc = _CACHE[key]

    shared = {
        "sx": sx_cm, "w0t": w0t, "oht4": oht4, "augr": augr_flat,
        "wcol": wcol_flat, "ohwn": ohwn_flat, "desc": desc,
    }
    in_maps = []
    for i in range(N_CORES):
        im = dict(shared)
        im["qx"] = np.ascontiguousarray(qx_cm[:, QL * i:QL * (i + 1), :])
        im["ycmp"] = np.ascontiguousarray(ycmp_all[i])
        in_maps.append(im)

    res = run_bass_kernel_spmd(nc, in_maps, core_ids=list(range(N_CORES)))
    global LAST_RESULT
    LAST_RESULT = res
    rew = np.concatenate([r["rew"].reshape(-1) for r in res.results])[:Q]
    return rew.astype(np.int32)


LAST_RESULT = None



# revision 25
# speedup vs baseline: 1.6338x; 1.6338x over previous
"""Trainium2 Bass kernel for nn_NegativeSoftmax (few-shot episode adaptation).

Math: the adapted weight W_t stays in span{W0 rows, pooled support features}:
    W_t[c,:] = a_t * W0[c,:] + sum_s B_t[c,s] * sxp[s,:]
where a_t is a data-independent scalar sequence (weight decay on the W0
component) and B_t [25,5] evolves via the SGD+momentum recurrence driven by
per-step softmax gradients.  The z.min subtraction in the reference has zero
gradient (softmax-gradient rows sum to 0) and doesn't move argmax, so it is
dropped.  The 700 sequential steps then only need [25]-sized linear algebra.

The 700-step loop is latency-bound (cross-engine dependency chain), so the
whole softmax tail runs on the Vector engine via custom DVE ops, with exp
computed as a clamped cubic polynomial + 7 squarings (e^x = p(x/128)^128,
|rel err| < 2e-5 for the softmax-relevant range after row-max subtraction):

The step recurrence is split so every instruction carries RAW deps from at
most ONE other engine (a second semaphore would force a SEQ-blocking
EventSemaphore):
    D'_t = wcol_t*(P_t/ssum_t - OH)      (nonlinear part incl. the one-hot)
    B_{t+1} = CWD1*B_t - MOM*B_{t-1} + D'_t
    y_t = ka@aug_t + kbC@B_{t-1} + kbM@B_{t-2} + kb@D'_{t-1}
          (4 PSUM-accum matmuls; only the D' one is chain-serial)
Per step on the chain (K pre-scaled by SCALE/128):
  MM_D (PE) -> DVE NS_MAXCOPY (y PSUM->SBUF + rowmax accum)
  -> DVE NS_EXPPOLY: q = relu(1 + u + u^2*(1/2 + u/6)), u = ysb - nmax
  -> DVE NS_SQ7SUM: P = q^128 (7 squarings), ssum = rowsum(P) (accum)
  -> DVE reciprocal_approx_fast (scalar op: zero ack latency)
  -> DVE NS_DFIN: D' = ((P*rs) - OH)*wcol  (+dummy accum: sem fires ~60ns
     earlier via the accum-read aux instruction, which skips the write-ack)
B materialization (two stt ops) rides the DVE gaps off the chain; query
pooling runs on the otherwise-idle Pool engine DURING the loop.

Final scoring:  scores ~ qs.T@B_T + 25*a_T*q0.T  -> argmax -> compare labels.

Distribution: the sequential loop is replicated on all 8 cores; each core
pools + scores its own 1/8 slice of the 600 queries (the only big data).
Outputs are int32 rewards gathered on host.
"""

import numpy as np

SCALE, MARGIN, LR, MOM, DAMP, WD = 10.0, 0.4, 1.01, 0.9, 0.9, 1e-3
N_CORES = 8
KPOW = 7                      # e^x = p(x/2^KPOW)^(2^KPOW)
KDIV = float(2 ** KPOW)

_CACHE = {}
_OPS = {}


def _register_dve_ops():
    """Register the four custom DVE ops used by the adaptation loop.

    Appends to concourse.dve_ops.OPS (rows 17+ of the 5-bit opcode table,
    built-ins occupy 1..16) and keeps CUSTOM_DVE_SPECS/_SUB_OPCODE_FOR_NAME
    in sync so codegen + table-gen + CoreSim all resolve the new names."""
    global _OPS
    if _OPS:
        return _OPS
    import operator

    import concourse.dve_ops as dve_ops
    from concourse.dve_spec import (
        AluOp, Bin, C0, C1, C2, MaxNeg, One, Spec, Src0, Src1, Zero,
        _has_src1, lower, maxx, relu, sq,
    )
    from concourse.dve_uop import DveOpSpec

    def ref_maxcopy(in0, in1, c0, c1, c2):
        a = np.asarray(in0, np.float32)
        return a, a.reshape(a.shape[0], -1).max(axis=-1, keepdims=True)

    def ref_exppoly(in0, in1, c0, c1, c2):
        a = np.asarray(in0, np.float32)
        u = (a - c0).astype(np.float32)
        u2 = (u * u).astype(np.float32)
        m = (np.float32(c2) * u).astype(np.float32)
        n = (m + c1).astype(np.float32)
        o = (n * u2).astype(np.float32)
        p = ((o + u).astype(np.float32) + np.float32(1.0)).astype(np.float32)
        return np.maximum(np.nan_to_num(p, nan=0.0), np.float32(0.0))

    def ref_sq7sum(in0, in1, c0, c1, c2):
        p = np.asarray(in0, np.float32)
        for _ in range(KPOW):
            p = (p * p).astype(np.float32)
        return p, p.reshape(p.shape[0], -1).sum(axis=-1, keepdims=True)

    def ref_dfin(in0, in1, c0, c1, c2):
        a = np.asarray(in0, np.float32)
        r = (((a * c0).astype(np.float32) - in1).astype(np.float32)
             * c1).astype(np.float32)
        return r, r.reshape(r.shape[0], -1).sum(axis=-1, keepdims=True)

    _u = Src0 - C0
    _u2 = _u * _u
    _poly = relu(((C2 * _u) + C1) * _u2 + _u + One)
    _q = Src0
    for _ in range(KPOW):
        _q = sq(_q)

    specs = [
        ("NS_MAXCOPY",
         Spec(body=Bin(AluOp.ADD, Src0, Zero), accum=maxx, accum_init=MaxNeg,
              reference=ref_maxcopy)),
        ("NS_EXPPOLY", Spec(body=_poly, reference=ref_exppoly)),
        ("NS_SQ7SUM",
         Spec(body=_q, accum=operator.add, accum_init=Zero,
              reference=ref_sq7sum)),
        ("NS_DFIN",
         Spec(body=((Src0 * C0) - Src1) * C1, accum=operator.add,
              accum_init=Zero, reference=ref_dfin)),
    ]

    existing = {op.name for op in dve_ops.OPS}
    for name, spec in specs:
        if name in existing:
            _OPS[name] = next(o for o in dve_ops.OPS if o.name == name)
            continue
        row = dve_ops._CUSTOM_DVE_ROW_BASE + len(dve_ops.OPS)
        assert row < 0x20, "5-bit opcode row overflow"
        shas = {}
        for ver in ("v3", "v4"):
            try:
                uops = lower(spec, ver=ver)
                shas[ver] = DveOpSpec(
                    name=name, opcode=row, uops=uops, rd1_en=_has_src1(spec)
                ).sha(ver)
            except ValueError:
                pass
        op = dve_ops.DveOp(name, spec, subdim=False, uops_sha=shas)
        dve_ops.OPS.append(op)
        dve_ops.CUSTOM_DVE_SPECS[name] = spec
        dve_ops._SUB_OPCODE_FOR_NAME[name] = row
        _OPS[name] = op
    return _OPS


def _host_a_seq(T):
    a, va = np.float32(1.0), np.float32(0.0)
    seq = [np.float32(a)]
    for t in range(T):
        d = np.float32(WD) * a
        va = d if t == 0 else np.float32(MOM) * va + np.float32(1.0 - DAMP) * d
        a = a - np.float32(LR) * va
        seq.append(np.float32(a))
    return np.asarray(seq, np.float32)


def _build_program(T, QL, n_cls, S, C, qred_chunk=2):
    import concourse.bacc as bacc
    import concourse.mybir as mybir
    import concourse.tile as tile

    ops = _register_dve_ops()

    f32 = mybir.dt.float32
    i32 = mybir.dt.int32
    NT = C // 128   # channel tiles
    NB = n_cls      # block width
    NA = 2 * n_cls  # aug rows (a_t*I ; I)
    NC = S + NA     # combined contraction rows

    nc = bacc.Bacc("TRN2", target_bir_lowering=False, name="negsoftmax")
    d_qx = nc.dram_tensor("qx", [C, QL, 25], f32, kind="ExternalInput")
    d_sx = nc.dram_tensor("sx", [C, S, 25], f32, kind="ExternalInput")
    d_w0t = nc.dram_tensor("w0t", [C, n_cls], f32, kind="ExternalInput")
    d_ohts = nc.dram_tensor("ohts", [n_cls, S], f32, kind="ExternalInput")
    # bcomb aug rows for every step t=0..T: [a_t*I ; I] blocks
    d_augr = nc.dram_tensor("augr", [NA, NB * (T + 1)], f32, kind="ExternalInput")
    d_wcol = nc.dram_tensor("wcol", [S, T], f32, kind="ExternalInput")
    d_ohp = nc.dram_tensor("ohp", [S, n_cls], f32, kind="ExternalInput")
    d_ycmp = nc.dram_tensor("ycmp", [QL, 1], f32, kind="ExternalInput")
    d_desc = nc.dram_tensor("desc", [QL, n_cls], f32, kind="ExternalInput")
    d_i5 = nc.dram_tensor("i5", [n_cls, n_cls], f32, kind="ExternalInput")
    d_rew = nc.dram_tensor("rew", [QL, 1], i32, kind="ExternalOutput")

    AX = mybir.AxisListType.X
    OP = mybir.AluOpType
    CWD1 = float(1.0 + MOM - LR * (1.0 - DAMP) * WD)   # t>=1 B_t coefficient

    with tile.TileContext(nc) as tc:
        with (
            tc.tile_pool(name="persist", bufs=1) as pp,
            tc.tile_pool(name="sraw", bufs=6) as sraw_pool,
            tc.tile_pool(name="step", bufs=6) as sp,
            tc.tile_pool(name="dbuf", bufs=6) as dp,
            tc.tile_pool(name="vbuf", bufs=6) as vp,
            tc.tile_pool(name="psum", bufs=2, space="PSUM") as psp,
            tc.tile_pool(name="psum_keep", bufs=1, space="PSUM") as pkp,
        ):
            # ---------------- persistent tiles ----------------
            # loop matmul lhsT split at the 32-partition alignment rule:
            #   kb [25,25] (B/h/D rows), ka [10,25] (aug rows), both scaled
            kb = pp.tile([S, S], f32)
            kbc = pp.tile([S, S], f32)
            kbm = pp.tile([S, S], f32)
            ka = pp.tile([NA, S], f32)
            # bta[:, NB*t:...] = [a_t*I ; I] ; btb column t = B_t (Pool-written)
            bta = pp.tile([NA, NB * (T + 1)], f32)
            btb = pp.tile([S, NB * (T + 1)], f32)
            wcol_sb = pp.tile([S, T], f32)
            oh_sb = pp.tile([S, n_cls], f32)
            v0_sb = pp.tile([S, NB], f32)
            ycmp_sb = pp.tile([QL, 1], f32)
            desc_sb = pp.tile([QL, n_cls], f32)
            i5_sb = pp.tile([n_cls, n_cls], f32)
            sxsum = pp.tile([128, NT * S], f32)    # pooled (sum) support, c-major
            w0t_sb = pp.tile([128, NT * n_cls], f32)
            qxraw = pp.tile([128, NT * QL * 25], f32)  # raw query features
            qxsum = pp.tile([128, NT * QL], f32)
            sq = pp.tile([S, QL], f32)             # qsrawT   (scoring lhsT)
            sq0 = pp.tile([n_cls, QL], f32)        # 25*a_T*q0rawT

            # ---------------- support pooling + K/z0 ----------------
            nc.scalar.dma_start(
                w0t_sb[:].rearrange("p (j c) -> p j c", j=NT),
                d_w0t[:].rearrange("(j p) c -> p j c", p=128))
            dma_engs = [nc.sync, nc.scalar, nc.gpsimd]
            for j in range(NT):
                eng = dma_engs[j % 3]
                sxr = sraw_pool.tile([128, S, 25], f32, tag="sxr")
                eng.dma_start(sxr[:], d_sx[128 * j:128 * (j + 1)])
                nc.vector.tensor_reduce(
                    out=sxsum[:, S * j:S * (j + 1)], in_=sxr[:], axis=AX, op=OP.add)

            # small loop-constant DMAs (needed only by loop start ~20us in)
            nc.sync.dma_start(wcol_sb[:], d_wcol[:])
            nc.sync.dma_start(oh_sb[:], d_ohp[:])
            nc.sync.dma_start(bta[:], d_augr[:])
            nc.sync.dma_start(ka[n_cls:NA, :], d_ohts[:])
            nc.sync.dma_start(ycmp_sb[:], d_ycmp[:])
            nc.sync.dma_start(desc_sb[:], d_desc[:])
            nc.sync.dma_start(i5_sb[:], d_i5[:])
            nc.vector.memset(btb[:, 0:NB], 0.0)
            nc.vector.memset(v0_sb[:], 0.0)

            kraw = pkp.tile([S, S], f32, tag="kraw")
            z0t = pkp.tile([n_cls, S], f32, tag="z0t")
            for j in range(NT):
                nc.tensor.matmul(
                    kraw[:], sxsum[:, S * j:S * (j + 1)], sxsum[:, S * j:S * (j + 1)],
                    start=(j == 0), stop=(j == NT - 1))
            for j in range(NT):
                nc.tensor.matmul(
                    z0t[:], w0t_sb[:, n_cls * j:n_cls * (j + 1)],
                    sxsum[:, S * j:S * (j + 1)],
                    start=(j == 0), stop=(j == NT - 1))
            kb_copy = nc.scalar.mul(kb[:], kraw[:], SCALE / 625.0 / KDIV)
            nc.scalar.mul(kbc[:], kraw[:], SCALE / 625.0 / KDIV * CWD1)
            nc.scalar.mul(kbm[:], kraw[:], -MOM * SCALE / 625.0 / KDIV)
            nc.scalar.mul(ka[0:n_cls, :], z0t[:], SCALE / 25.0 / KDIV)

            # query raw loads: gate on kb so they don't steal DMA bandwidth
            # from the prologue (SP runs ahead and would start them at t=0)
            from concourse.tile import add_dep_helper
            for j in range(NT):
                dq = nc.sync.dma_start(
                    qxraw[:, QL * 25 * j:QL * 25 * (j + 1)],
                    d_qx[128 * j:128 * (j + 1)])
                add_dep_helper(dq.ins, kb_copy.ins, sync=True,
                               reason="defer qx DMA past prologue")

            # query-pooling work list (drained one chunk per loop step)
            qv = qxraw[:].rearrange("p (j q s) -> p j q s", j=NT, q=QL)
            POOL_T0 = 60
            pool_chunks = []
            for j in range(NT):
                for q0 in range(0, QL, qred_chunk):
                    pool_chunks.append((j, q0, min(q0 + qred_chunk, QL)))

            # ---------------- the T-step adaptation loop ----------------
            # y_t = ka@aug_t + kbC@B_{t-1} + kbM@B_{t-2} + kb@D'_{t-1}
            # (PSUM accumulation; only the D' matmul is chain-serial)
            # D'_t = wcol_t*(P_t*rs_t - OH)   (includes the OHWn term)
            # B_{t+1} = CWD1*B_t - MOM*B_{t-1} + D'_t
            d_prev = None      # D'_{t-1} tile
            for t in range(T):
                y10 = psp.tile([S, n_cls], f32, tag="y10")
                nc.tensor.matmul(y10[:], ka[:], bta[:, NB * t:NB * (t + 1)],
                                 start=True, stop=(t == 0))
                if t >= 2:
                    nc.tensor.matmul(y10[:], kbc[:],
                                     btb[:, NB * (t - 1):NB * t],
                                     start=False, stop=False)
                if t >= 3:
                    nc.tensor.matmul(y10[:], kbm[:],
                                     btb[:, NB * (t - 2):NB * (t - 1)],
                                     start=False, stop=False)
                if t >= 1:
                    nc.tensor.matmul(y10[:], kb[:], d_prev[:],
                                     start=False, stop=True)

                ysb = sp.tile([S, n_cls], f32, tag="ysb")
                nmax = sp.tile([S, 1], f32, tag="nmax")
                nc.vector._custom_dve(
                    ops["NS_MAXCOPY"], out=ysb[:], in0=y10[:], accum_out=nmax[:])
                pq = sp.tile([S, n_cls], f32, tag="pq")
                nc.vector._custom_dve(
                    ops["NS_EXPPOLY"], out=pq[:], in0=ysb[:],
                    s0=nmax[:, 0:1], s1=0.5, imm2=1.0 / 6.0)
                p = sp.tile([S, n_cls], f32, tag="p")
                ssum = sp.tile([S, 1], f32, tag="ssum")
                nc.vector._custom_dve(
                    ops["NS_SQ7SUM"], out=p[:], in0=pq[:], accum_out=ssum[:])
                rs = sp.tile([S, 1], f32, tag="rs")
                nc.vector.reciprocal_approx_fast(rs[:], ssum[:])
                dt = dp.tile([S, NB], f32, tag="d")
                junk = sp.tile([S, 1], f32, tag="junk")
                nc.vector._custom_dve(
                    ops["NS_DFIN"], out=dt[:], in0=p[:], in1=oh_sb[:],
                    s0=rs[:, 0:1], s1=wcol_sb[:, t:t + 1], accum_out=junk[:])

                # B_{t+1} = CWD1*v_t + D'_t with v_t = B_t - (M/CWD1)*B_{t-1}
                # on Pool: btb is only read by PE matmuls and Pool itself, so
                # both ops carry at most one cross-engine (DVE: D') semaphore
                b_next = btb[:, NB * (t + 1):NB * (t + 2)]
                if t == 0:
                    v_in = v0_sb[:]
                else:
                    vt = vp.tile([S, NB], f32, tag="v")
                    nc.vector.scalar_tensor_tensor(
                        out=vt[:], in0=btb[:, NB * (t - 1):NB * t],
                        scalar=-MOM / CWD1, in1=btb[:, NB * t:NB * (t + 1)],
                        op0=OP.mult, op1=OP.add)
                    v_in = vt[:]
                bupd = nc.vector.scalar_tensor_tensor(
                    out=b_next, in0=v_in, scalar=CWD1, in1=dt[:],
                    op0=OP.mult, op1=OP.add)
                d_prev = dt

                # one small query-pooling chunk per step, placed in program
                # order so it lands in the MM-wait window (no clustering)
                if pool_chunks and t >= POOL_T0:
                    j, q0, q1 = pool_chunks.pop(0)
                    red = nc.vector.tensor_reduce(
                        out=qxsum[:, QL * j + q0:QL * j + q1],
                        in_=qv[:, j, q0:q1], axis=AX, op=OP.add)
                    add_dep_helper(red.ins, bupd.ins, sync=False,
                                   reason="pin pooling chunk to its loop slot")

            # ------------- leftover query-pooling chunks -------
            for j, q0, q1 in pool_chunks:
                nc.vector.tensor_reduce(
                    out=qxsum[:, QL * j + q0:QL * j + q1],
                    in_=qv[:, j, q0:q1], axis=AX, op=OP.add)
            qst = pkp.tile([S, QL], f32, tag="qst")
            q0t = pkp.tile([n_cls, QL], f32, tag="q0t")
            for j in range(NT):
                nc.tensor.matmul(
                    qst[:], sxsum[:, S * j:S * (j + 1)], qxsum[:, QL * j:QL * (j + 1)],
                    start=(j == 0), stop=(j == NT - 1))
            for j in range(NT):
                nc.tensor.matmul(
                    q0t[:], w0t_sb[:, n_cls * j:n_cls * (j + 1)],
                    qxsum[:, QL * j:QL * (j + 1)],
                    start=(j == 0), stop=(j == NT - 1))
            a_T = float(_host_a_seq(T)[T])
            nc.scalar.mul(sq[:], qst[:], 1.0)
            nc.scalar.mul(sq0[:], q0t[:], 25.0 * a_T)

            scores = pkp.tile([QL, n_cls], f32, tag="scores")
            nc.tensor.matmul(scores[:], sq[:], btb[:, NB * T:NB * (T + 1)],
                             start=True, stop=False)
            nc.tensor.matmul(scores[:], sq0[:], i5_sb[:],
                             start=False, stop=True)

            mx = pp.tile([QL, 1], f32)
            eq = pp.tile([QL, n_cls], f32)
            vv = pp.tile([QL, n_cls], f32)
            rr = pp.tile([QL, 1], f32)
            okf = pp.tile([QL, 1], f32)
            oki = pp.tile([QL, 1], i32)
            nc.vector.tensor_reduce(out=mx[:], in_=scores[:], axis=AX, op=OP.max)
            nc.vector.tensor_scalar(
                out=eq[:], in0=scores[:], scalar1=mx[:, 0:1], scalar2=None,
                op0=OP.is_equal)
            nc.vector.tensor_mul(vv[:], eq[:], desc_sb[:])
            nc.vector.tensor_reduce(out=rr[:], in_=vv[:], axis=AX, op=OP.max)
            nc.vector.tensor_scalar(
                out=okf[:], in0=rr[:], scalar1=ycmp_sb[:, 0:1], scalar2=None,
                op0=OP.is_equal)
            nc.vector.tensor_copy(oki[:], okf[:])
            nc.sync.dma_start(d_rew[:], oki[:])

    nc.compile()
    return nc


def kernel(support_xf, support_y, query_xf, query_y, n_way, k_shot,
           batch_ids, batch_mask, weight_init, **_unused):
    import os
    # the axon NTFF-trace hook module isn't shipped in this container;
    # make sure a stray BASS_TRACE can't route us into that import
    os.environ["BASS_NEVER_TRACE"] = "1"
    from concourse.bass_utils import run_bass_kernel_spmd

    f32 = np.float32
    support_xf = np.ascontiguousarray(np.asarray(support_xf, f32))
    query_xf = np.ascontiguousarray(np.asarray(query_xf, f32))
    W0 = np.asarray(weight_init, f32)
    sy = np.asarray(support_y).reshape(-1).astype(np.int64)
    qy = np.asarray(query_y).reshape(-1).astype(np.int64)
    ids = np.asarray(batch_ids)
    mk = np.asarray(batch_mask)

    n_cls = W0.shape[0]
    S = support_xf.shape[1]
    C = support_xf.shape[2]
    T = ids.shape[0]
    Q = query_xf.shape[1]
    QL = (Q + N_CORES - 1) // N_CORES

    # ---- host preprocessing (layout + index/mask-derived constants) ----
    sx_cm = support_xf.reshape(S, C, 25).transpose(1, 0, 2).copy()   # [C,S,25]
    qx_cm = query_xf.reshape(Q, C, 25).transpose(1, 0, 2)            # [C,Q,25]
    if QL * N_CORES != Q:
        pad = QL * N_CORES - Q
        qx_cm = np.concatenate([qx_cm, np.zeros((C, pad, 25), f32)], axis=1)
        qy = np.concatenate([qy, np.zeros(pad, np.int64)])

    a_seq = _host_a_seq(T)
    I5 = np.eye(n_cls, dtype=f32)
    # augr rows: [a_t*I ; I] for t = 0..T, laid out as [2*n_cls, n_cls*(T+1)]
    augr = np.empty((T + 1, 2 * n_cls, n_cls), f32)
    augr[:, :n_cls, :] = a_seq[:, None, None] * I5[None]
    augr[:, n_cls:, :] = I5[None]
    augr_flat = augr.transpose(1, 0, 2).reshape(2 * n_cls, n_cls * (T + 1)).copy()

    m = mk.astype(f32)
    cnt = m.sum(1)
    w0w = np.zeros((T, S), f32)
    for b in range(ids.shape[1]):
        np.add.at(w0w, (np.arange(T), ids[:, b]), m[:, b])
    w0w /= cnt[:, None]
    kk = np.full(T, 1.0 - DAMP, f32)
    kk[0] = 1.0
    wcol = (-LR * kk[:, None] * SCALE * w0w).astype(f32)             # [T,S]
    OH = I5[sy]                                                      # [S,5]
    wcol_flat = wcol.T.copy()                                        # [S,T]
    ohts = (-4.0 / KDIV * OH.T).copy()                               # [5,S]
    w0t = W0.T.copy()                                                # [C,5]
    desc = np.broadcast_to(
        np.arange(n_cls, 0, -1, dtype=f32)[None, :], (QL, n_cls)).copy()
    ycmp_all = (f32(n_cls) - qy.astype(f32)).reshape(N_CORES, QL, 1)

    key = (T, QL, n_cls, S, C)
    if key not in _CACHE:
        _CACHE[key] = _build_program(T, QL, n_cls, S, C)
    nc = _CACHE[key]

    shared = {
        "sx": sx_cm, "w0t": w0t, "ohts": ohts, "augr": augr_flat,
        "wcol": wcol_flat, "ohp": OH, "desc": desc, "i5": I5,
    }
    in_maps = []
    for i in range(N_CORES):
        im = dict(shared)
        im["qx"] = np.ascontiguousarray(qx_cm[:, QL * i:QL * (i + 1), :])
        im["ycmp"] = np.ascontiguousarray(ycmp_all[i])
        in_maps.append(im)

    res = run_bass_kernel_spmd(nc, in_maps, core_ids=list(range(N_CORES)))
    global LAST_RESULT
    LAST_RESULT = res
    rew = np.concatenate([r["rew"].reshape(-1) for r in res.results])[:Q]
    return rew.astype(np.int32)


LAST_RESULT = None


# revision 30
# speedup vs baseline: 1.6359x; 1.0013x over previous
"""Trainium2 Bass kernel for nn_NegativeSoftmax (few-shot episode adaptation).

Math: the adapted weight W_t stays in span{W0 rows, pooled support features}:
    W_t[c,:] = a_t * W0[c,:] + sum_s B_t[c,s] * sxp[s,:]
where a_t is a data-independent scalar sequence (weight decay on the W0
component) and B_t [25,5] evolves via the SGD+momentum recurrence driven by
per-step softmax gradients.  The z.min subtraction in the reference has zero
gradient (softmax-gradient rows sum to 0) and doesn't move argmax, so it is
dropped.  The 700 sequential steps then only need [25]-sized linear algebra.

The 700-step loop is latency-bound (cross-engine dependency chain), so the
whole softmax tail runs on the Vector engine via custom DVE ops, with exp
computed as a clamped cubic polynomial + 7 squarings (e^x = p(x/128)^128,
|rel err| < 2e-5 for the softmax-relevant range after row-max subtraction):

The step recurrence is split so every instruction carries RAW deps from at
most ONE other engine (a second semaphore would force a SEQ-blocking
EventSemaphore):
    D'_t = wcol_t*(P_t/ssum_t - OH)      (nonlinear part incl. the one-hot)
    B_{t+1} = CWD1*B_t - MOM*B_{t-1} + D'_t
    y_t = ka@aug_t + kbC@B_{t-1} + kbM@B_{t-2} + kb@D'_{t-1}
          (4 PSUM-accum matmuls; only the D' one is chain-serial)
Per step on the chain (K pre-scaled by SCALE/128):
  MM_D (PE) -> DVE NS_MAXCOPY (y PSUM->SBUF + rowmax accum)
  -> DVE NS_EXPPOLY: q = relu(1 + u + u^2*(1/2 + u/6)), u = ysb - nmax
  -> DVE NS_SQ7SUM: P = q^128 (7 squarings), ssum = rowsum(P) (accum)
  -> DVE reciprocal_approx_fast (scalar op: zero ack latency)
  -> DVE NS_DFIN: D' = ((P*rs) - OH)*wcol  (+dummy accum: sem fires ~60ns
     earlier via the accum-read aux instruction, which skips the write-ack)
B materialization (two stt ops) rides the DVE gaps off the chain; query
pooling runs on the otherwise-idle Pool engine DURING the loop.

Final scoring:  scores ~ qs.T@B_T + 25*a_T*q0.T  -> argmax -> compare labels.

Distribution: the sequential loop is replicated on all 8 cores; each core
pools + scores its own 1/8 slice of the 600 queries (the only big data).
Outputs are int32 rewards gathered on host.
"""

import numpy as np

SCALE, MARGIN, LR, MOM, DAMP, WD = 10.0, 0.4, 1.01, 0.9, 0.9, 1e-3
N_CORES = 8
KPOW = 7                      # e^x = p(x/2^KPOW)^(2^KPOW)
KDIV = float(2 ** KPOW)

_CACHE = {}
_OPS = {}


def _register_dve_ops():
    """Register the four custom DVE ops used by the adaptation loop.

    Appends to concourse.dve_ops.OPS (rows 17+ of the 5-bit opcode table,
    built-ins occupy 1..16) and keeps CUSTOM_DVE_SPECS/_SUB_OPCODE_FOR_NAME
    in sync so codegen + table-gen + CoreSim all resolve the new names."""
    global _OPS
    if _OPS:
        return _OPS
    import operator

    import concourse.dve_ops as dve_ops
    from concourse.dve_spec import (
        AluOp, Bin, C0, C1, C2, MaxNeg, One, Spec, Src0, Src1, Zero,
        _has_src1, lower, maxx, relu, sq,
    )
    from concourse.dve_uop import DveOpSpec

    def ref_maxcopy(in0, in1, c0, c1, c2):
        a = np.asarray(in0, np.float32)
        return a, a.reshape(a.shape[0], -1).max(axis=-1, keepdims=True)

    def ref_exppoly(in0, in1, c0, c1, c2):
        a = np.asarray(in0, np.float32)
        u = (a - c0).astype(np.float32)
        u2 = (u * u).astype(np.float32)
        m = (np.float32(c2) * u).astype(np.float32)
        n = (m + c1).astype(np.float32)
        o = (n * u2).astype(np.float32)
        p = ((o + u).astype(np.float32) + np.float32(1.0)).astype(np.float32)
        return np.maximum(np.nan_to_num(p, nan=0.0), np.float32(0.0))

    def ref_sq7sum(in0, in1, c0, c1, c2):
        p = np.asarray(in0, np.float32)
        for _ in range(KPOW):
            p = (p * p).astype(np.float32)
        return p, p.reshape(p.shape[0], -1).sum(axis=-1, keepdims=True)

    def ref_dfin(in0, in1, c0, c1, c2):
        a = np.asarray(in0, np.float32)
        r = (((a * c0).astype(np.float32) - in1).astype(np.float32)
             * c1).astype(np.float32)
        return r, r.reshape(r.shape[0], -1).sum(axis=-1, keepdims=True)

    _u = Src0 - C0
    _u2 = _u * _u
    _poly = relu(((C2 * _u) + C1) * _u2 + _u + One)
    _q = Src0
    for _ in range(KPOW):
        _q = sq(_q)

    specs = [
        ("NS_MAXCOPY",
         Spec(body=Bin(AluOp.ADD, Src0, Zero), accum=maxx, accum_init=MaxNeg,
              reference=ref_maxcopy)),
        ("NS_EXPPOLY", Spec(body=_poly, reference=ref_exppoly)),
        ("NS_SQ7SUM",
         Spec(body=_q, accum=operator.add, accum_init=Zero,
              reference=ref_sq7sum)),
        ("NS_DFIN",
         Spec(body=((Src0 * C0) - Src1) * C1, accum=operator.add,
              accum_init=Zero, reference=ref_dfin)),
    ]

    existing = {op.name for op in dve_ops.OPS}
    for name, spec in specs:
        if name in existing:
            _OPS[name] = next(o for o in dve_ops.OPS if o.name == name)
            continue
        row = dve_ops._CUSTOM_DVE_ROW_BASE + len(dve_ops.OPS)
        assert row < 0x20, "5-bit opcode row overflow"
        shas = {}
        for ver in ("v3", "v4"):
            try:
                uops = lower(spec, ver=ver)
                shas[ver] = DveOpSpec(
                    name=name, opcode=row, uops=uops, rd1_en=_has_src1(spec)
                ).sha(ver)
            except ValueError:
                pass
        op = dve_ops.DveOp(name, spec, subdim=False, uops_sha=shas)
        dve_ops.OPS.append(op)
        dve_ops.CUSTOM_DVE_SPECS[name] = spec
        dve_ops._SUB_OPCODE_FOR_NAME[name] = row
        _OPS[name] = op
    return _OPS


def _host_a_seq(T):
    a, va = np.float32(1.0), np.float32(0.0)
    seq = [np.float32(a)]
    for t in range(T):
        d = np.float32(WD) * a
        va = d if t == 0 else np.float32(MOM) * va + np.float32(1.0 - DAMP) * d
        a = a - np.float32(LR) * va
        seq.append(np.float32(a))
    return np.asarray(seq, np.float32)


def _build_program(T, QL, n_cls, S, C, qred_chunk=2):
    import concourse.bacc as bacc
    import concourse.mybir as mybir
    import concourse.tile as tile

    ops = _register_dve_ops()

    f32 = mybir.dt.float32
    i32 = mybir.dt.int32
    NT = C // 128   # channel tiles
    NB = n_cls      # block width
    NA = 2 * n_cls  # aug rows (a_t*I ; I)
    NC = S + NA     # combined contraction rows

    nc = bacc.Bacc("TRN2", target_bir_lowering=False, name="negsoftmax")
    d_qx = nc.dram_tensor("qx", [C, QL, 25], f32, kind="ExternalInput")
    d_sx = nc.dram_tensor("sx", [C, S, 25], f32, kind="ExternalInput")
    d_w0t = nc.dram_tensor("w0t", [C, n_cls], f32, kind="ExternalInput")
    d_ohts = nc.dram_tensor("ohts", [n_cls, S], f32, kind="ExternalInput")
    # bcomb aug rows for every step t=0..T: [a_t*I ; I] blocks
    d_augr = nc.dram_tensor("augr", [NA, NB * (T + 1)], f32, kind="ExternalInput")
    d_wcol = nc.dram_tensor("wcol", [S, T], f32, kind="ExternalInput")
    d_ohp = nc.dram_tensor("ohp", [S, n_cls], f32, kind="ExternalInput")
    d_ycmp = nc.dram_tensor("ycmp", [QL, 1], f32, kind="ExternalInput")
    d_desc = nc.dram_tensor("desc", [QL, n_cls], f32, kind="ExternalInput")
    d_i5 = nc.dram_tensor("i5", [n_cls, n_cls], f32, kind="ExternalInput")
    d_rew = nc.dram_tensor("rew", [QL, 1], i32, kind="ExternalOutput")

    AX = mybir.AxisListType.X
    OP = mybir.AluOpType
    CWD1 = float(1.0 + MOM - LR * (1.0 - DAMP) * WD)   # t>=1 B_t coefficient

    with tile.TileContext(nc) as tc:
        with (
            tc.tile_pool(name="persist", bufs=1) as pp,
            tc.tile_pool(name="sraw", bufs=6) as sraw_pool,
            tc.tile_pool(name="step", bufs=6) as sp,
            tc.tile_pool(name="dbuf", bufs=6) as dp,
            tc.tile_pool(name="vbuf", bufs=6) as vp,
            tc.tile_pool(name="psum", bufs=2, space="PSUM") as psp,
            tc.tile_pool(name="psum_keep", bufs=1, space="PSUM") as pkp,
        ):
            # ---------------- persistent tiles ----------------
            # loop matmul lhsT split at the 32-partition alignment rule:
            #   kb [25,25] (B/h/D rows), ka [10,25] (aug rows), both scaled
            kb = pp.tile([S, S], f32)
            kbc = pp.tile([S, S], f32)
            kbm = pp.tile([S, S], f32)
            ka = pp.tile([NA, S], f32)
            # bta[:, NB*t:...] = [a_t*I ; I] ; btb column t = B_t (Pool-written)
            bta = pp.tile([NA, NB * (T + 1)], f32)
            btb = pp.tile([S, NB * (T + 1)], f32)
            wcol_sb = pp.tile([S, T], f32)
            oh_sb = pp.tile([S, n_cls], f32)
            v0_sb = pp.tile([S, NB], f32)
            ycmp_sb = pp.tile([QL, 1], f32)
            desc_sb = pp.tile([QL, n_cls], f32)
            i5_sb = pp.tile([n_cls, n_cls], f32)
            sxsum = pp.tile([128, NT * S], f32)    # pooled (sum) support, c-major
            w0t_sb = pp.tile([128, NT * n_cls], f32)
            qxraw = pp.tile([128, NT * QL * 25], f32)  # raw query features
            qxsum = pp.tile([128, NT * QL], f32)
            sq = pp.tile([S, QL], f32)             # qsrawT   (scoring lhsT)
            sq0 = pp.tile([n_cls, QL], f32)        # 25*a_T*q0rawT

            # ---------------- support pooling + K/z0 ----------------
            nc.scalar.dma_start(
                w0t_sb[:].rearrange("p (j c) -> p j c", j=NT),
                d_w0t[:].rearrange("(j p) c -> p j c", p=128))
            dma_engs = [nc.sync, nc.scalar]
            for j in range(NT):
                eng = dma_engs[j % 2]
                sxr = sraw_pool.tile([128, S, 25], f32, tag="sxr")
                eng.dma_start(sxr[:], d_sx[128 * j:128 * (j + 1)])
                nc.vector.tensor_reduce(
                    out=sxsum[:, S * j:S * (j + 1)], in_=sxr[:], axis=AX, op=OP.add)

            # small loop-constant DMAs on the Pool ring (SWDGE), off the
            # sx critical path
            nc.gpsimd.dma_start(bta[:], d_augr[:])
            nc.gpsimd.dma_start(wcol_sb[:], d_wcol[:])
            nc.gpsimd.dma_start(oh_sb[:], d_ohp[:])
            nc.gpsimd.dma_start(ka[n_cls:NA, :], d_ohts[:])
            nc.gpsimd.dma_start(ycmp_sb[:], d_ycmp[:])
            nc.gpsimd.dma_start(desc_sb[:], d_desc[:])
            nc.gpsimd.dma_start(i5_sb[:], d_i5[:])
            nc.vector.memset(btb[:, 0:NB], 0.0)
            nc.vector.memset(v0_sb[:], 0.0)

            kraw = pkp.tile([S, S], f32, tag="kraw")
            z0t = pkp.tile([n_cls, S], f32, tag="z0t")
            for j in range(NT):
                nc.tensor.matmul(
                    kraw[:], sxsum[:, S * j:S * (j + 1)], sxsum[:, S * j:S * (j + 1)],
                    start=(j == 0), stop=(j == NT - 1))
            for j in range(NT):
                nc.tensor.matmul(
                    z0t[:], w0t_sb[:, n_cls * j:n_cls * (j + 1)],
                    sxsum[:, S * j:S * (j + 1)],
                    start=(j == 0), stop=(j == NT - 1))
            kb_copy = nc.scalar.mul(kb[:], kraw[:], SCALE / 625.0 / KDIV)
            nc.scalar.mul(kbc[:], kraw[:], SCALE / 625.0 / KDIV * CWD1)
            nc.scalar.mul(kbm[:], kraw[:], -MOM * SCALE / 625.0 / KDIV)
            nc.scalar.mul(ka[0:n_cls, :], z0t[:], SCALE / 25.0 / KDIV)

            # query raw loads: gate on kb so they don't steal DMA bandwidth
            # from the prologue (SP runs ahead and would start them at t=0)
            from concourse.tile import add_dep_helper
            for j in range(NT):
                dq = nc.sync.dma_start(
                    qxraw[:, QL * 25 * j:QL * 25 * (j + 1)],
                    d_qx[128 * j:128 * (j + 1)])
                add_dep_helper(dq.ins, kb_copy.ins, sync=True,
                               reason="defer qx DMA past prologue")

            # query-pooling work list (drained one chunk per loop step)
            qv = qxraw[:].rearrange("p (j q s) -> p j q s", j=NT, q=QL)
            POOL_T0 = 60
            pool_chunks = []
            for j in range(NT):
                for q0 in range(0, QL, qred_chunk):
                    pool_chunks.append((j, q0, min(q0 + qred_chunk, QL)))

            # ---------------- the T-step adaptation loop ----------------
            # y_t = ka@aug_t + kbC@B_{t-1} + kbM@B_{t-2} + kb@D'_{t-1}
            # (PSUM accumulation; only the D' matmul is chain-serial)
            # D'_t = wcol_t*(P_t*rs_t - OH)   (includes the OHWn term)
            # B_{t+1} = CWD1*B_t - MOM*B_{t-1} + D'_t
            d_prev = None      # D'_{t-1} tile
            for t in range(T):
                y10 = psp.tile([S, n_cls], f32, tag="y10")
                nc.tensor.matmul(y10[:], ka[:], bta[:, NB * t:NB * (t + 1)],
                                 start=True, stop=(t == 0))
                if t >= 2:
                    nc.tensor.matmul(y10[:], kbc[:],
                                     btb[:, NB * (t - 1):NB * t],
                                     start=False, stop=False)
                if t >= 3:
                    nc.tensor.matmul(y10[:], kbm[:],
                                     btb[:, NB * (t - 2):NB * (t - 1)],
                                     start=False, stop=False)
                if t >= 1:
                    nc.tensor.matmul(y10[:], kb[:], d_prev[:],
                                     start=False, stop=True)

                ysb = sp.tile([S, n_cls], f32, tag="ysb")
                nmax = sp.tile([S, 1], f32, tag="nmax")
                nc.vector._custom_dve(
                    ops["NS_MAXCOPY"], out=ysb[:], in0=y10[:], accum_out=nmax[:])
                pq = sp.tile([S, n_cls], f32, tag="pq")
                nc.vector._custom_dve(
                    ops["NS_EXPPOLY"], out=pq[:], in0=ysb[:],
                    s0=nmax[:, 0:1], s1=0.5, imm2=1.0 / 6.0)
                p = sp.tile([S, n_cls], f32, tag="p")
                ssum = sp.tile([S, 1], f32, tag="ssum")
                nc.vector._custom_dve(
                    ops["NS_SQ7SUM"], out=p[:], in0=pq[:], accum_out=ssum[:])
                rs = sp.tile([S, 1], f32, tag="rs")
                nc.vector.reciprocal_approx_fast(rs[:], ssum[:])
                dt = dp.tile([S, NB], f32, tag="d")
                junk = sp.tile([S, 1], f32, tag="junk")
                nc.vector._custom_dve(
                    ops["NS_DFIN"], out=dt[:], in0=p[:], in1=oh_sb[:],
                    s0=rs[:, 0:1], s1=wcol_sb[:, t:t + 1], accum_out=junk[:])

                # B_{t+1} = CWD1*v_t + D'_t with v_t = B_t - (M/CWD1)*B_{t-1}
                # on Pool: btb is only read by PE matmuls and Pool itself, so
                # both ops carry at most one cross-engine (DVE: D') semaphore
                b_next = btb[:, NB * (t + 1):NB * (t + 2)]
                if t == 0:
                    v_in = v0_sb[:]
                else:
                    vt = vp.tile([S, NB], f32, tag="v")
                    nc.vector.scalar_tensor_tensor(
                        out=vt[:], in0=btb[:, NB * (t - 1):NB * t],
                        scalar=-MOM / CWD1, in1=btb[:, NB * t:NB * (t + 1)],
                        op0=OP.mult, op1=OP.add)
                    v_in = vt[:]
                bupd = nc.vector.scalar_tensor_tensor(
                    out=b_next, in0=v_in, scalar=CWD1, in1=dt[:],
                    op0=OP.mult, op1=OP.add)
                d_prev = dt

                # one small query-pooling chunk per step, placed in program
                # order so it lands in the MM-wait window (no clustering)
                if pool_chunks and t >= POOL_T0:
                    j, q0, q1 = pool_chunks.pop(0)
                    red = nc.vector.tensor_reduce(
                        out=qxsum[:, QL * j + q0:QL * j + q1],
                        in_=qv[:, j, q0:q1], axis=AX, op=OP.add)
                    add_dep_helper(red.ins, bupd.ins, sync=False,
                                   reason="pin pooling chunk to its loop slot")

            # ------------- leftover query-pooling chunks -------
            for j, q0, q1 in pool_chunks:
                nc.vector.tensor_reduce(
                    out=qxsum[:, QL * j + q0:QL * j + q1],
                    in_=qv[:, j, q0:q1], axis=AX, op=OP.add)
            qst = pkp.tile([S, QL], f32, tag="qst")
            q0t = pkp.tile([n_cls, QL], f32, tag="q0t")
            for j in range(NT):
                nc.tensor.matmul(
                    qst[:], sxsum[:, S * j:S * (j + 1)], qxsum[:, QL * j:QL * (j + 1)],
                    start=(j == 0), stop=(j == NT - 1))
            for j in range(NT):
                nc.tensor.matmul(
                    q0t[:], w0t_sb[:, n_cls * j:n_cls * (j + 1)],
                    qxsum[:, QL * j:QL * (j + 1)],
                    start=(j == 0), stop=(j == NT - 1))
            a_T = float(_host_a_seq(T)[T])
            nc.scalar.mul(sq[:], qst[:], 1.0)
            nc.scalar.mul(sq0[:], q0t[:], 25.0 * a_T)

            scores = pkp.tile([QL, n_cls], f32, tag="scores")
            nc.tensor.matmul(scores[:], sq[:], btb[:, NB * T:NB * (T + 1)],
                             start=True, stop=False)
            nc.tensor.matmul(scores[:], sq0[:], i5_sb[:],
                             start=False, stop=True)

            mx = pp.tile([QL, 1], f32)
            eq = pp.tile([QL, n_cls], f32)
            vv = pp.tile([QL, n_cls], f32)
            rr = pp.tile([QL, 1], f32)
            okf = pp.tile([QL, 1], f32)
            oki = pp.tile([QL, 1], i32)
            nc.vector.tensor_reduce(out=mx[:], in_=scores[:], axis=AX, op=OP.max)
            nc.vector.tensor_scalar(
                out=eq[:], in0=scores[:], scalar1=mx[:, 0:1], scalar2=None,
                op0=OP.is_equal)
            nc.vector.tensor_mul(vv[:], eq[:], desc_sb[:])
            nc.vector.tensor_reduce(out=rr[:], in_=vv[:], axis=AX, op=OP.max)
            nc.vector.tensor_scalar(
                out=okf[:], in0=rr[:], scalar1=ycmp_sb[:, 0:1], scalar2=None,
                op0=OP.is_equal)
            nc.vector.tensor_copy(oki[:], okf[:])
            nc.sync.dma_start(d_rew[:], oki[:])

    nc.compile()
    return nc


def kernel(support_xf, support_y, query_xf, query_y, n_way, k_shot,
           batch_ids, batch_mask, weight_init, **_unused):
    import os
    # the axon NTFF-trace hook module isn't shipped in this container;
    # make sure a stray BASS_TRACE can't route us into that import
    os.environ["BASS_NEVER_TRACE"] = "1"
    from concourse.bass_utils import run_bass_kernel_spmd

    f32 = np.float32
    support_xf = np.ascontiguousarray(np.asarray(support_xf, f32))
    query_xf = np.ascontiguousarray(np.asarray(query_xf, f32))
    W0 = np.asarray(weight_init, f32)
    sy = np.asarray(support_y).reshape(-1).astype(np.int64)
    qy = np.asarray(query_y).reshape(-1).astype(np.int64)
    ids = np.asarray(batch_ids)
    mk = np.asarray(batch_mask)

    n_cls = W0.shape[0]
    S = support_xf.shape[1]
    C = support_xf.shape[2]
    T = ids.shape[0]
    Q = query_xf.shape[1]
    QL = (Q + N_CORES - 1) // N_CORES

    # ---- host preprocessing (layout + index/mask-derived constants) ----
    sx_cm = support_xf.reshape(S, C, 25).transpose(1, 0, 2).copy()   # [C,S,25]
    qx_cm = query_xf.reshape(Q, C, 25).transpose(1, 0, 2)            # [C,Q,25]
    if QL * N_CORES != Q:
        pad = QL * N_CORES - Q
        qx_cm = np.concatenate([qx_cm, np.zeros((C, pad, 25), f32)], axis=1)
        qy = np.concatenate([qy, np.zeros(pad, np.int64)])

    a_seq = _host_a_seq(T)
    I5 = np.eye(n_cls, dtype=f32)
    # augr rows: [a_t*I ; I] for t = 0..T, laid out as [2*n_cls, n_cls*(T+1)]
    augr = np.empty((T + 1, 2 * n_cls, n_cls), f32)
    augr[:, :n_cls, :] = a_seq[:, None, None] * I5[None]
    augr[:, n_cls:, :] = I5[None]
    augr_flat = augr.transpose(1, 0, 2).reshape(2 * n_cls, n_cls * (T + 1)).copy()

    m = mk.astype(f32)
    cnt = m.sum(1)
    w0w = np.zeros((T, S), f32)
    for b in range(ids.shape[1]):
        np.add.at(w0w, (np.arange(T), ids[:, b]), m[:, b])
    w0w /= cnt[:, None]
    kk = np.full(T, 1.0 - DAMP, f32)
    kk[0] = 1.0
    wcol = (-LR * kk[:, None] * SCALE * w0w).astype(f32)             # [T,S]
    OH = I5[sy]                                                      # [S,5]
    wcol_flat = wcol.T.copy()                                        # [S,T]
    ohts = (-4.0 / KDIV * OH.T).copy()                               # [5,S]
    w0t = W0.T.copy()                                                # [C,5]
    desc = np.broadcast_to(
        np.arange(n_cls, 0, -1, dtype=f32)[None, :], (QL, n_cls)).copy()
    ycmp_all = (f32(n_cls) - qy.astype(f32)).reshape(N_CORES, QL, 1)

    key = (T, QL, n_cls, S, C)
    if key not in _CACHE:
        _CACHE[key] = _build_program(T, QL, n_cls, S, C)
    nc = _CACHE[key]

    shared = {
        "sx": sx_cm, "w0t": w0t, "ohts": ohts, "augr": augr_flat,
        "wcol": wcol_flat, "ohp": OH, "desc": desc, "i5": I5,
    }
    in_maps = []
    for i in range(N_CORES):
        im = dict(shared)
        im["qx"] = np.ascontiguousarray(qx_cm[:, QL * i:QL * (i + 1), :])
        im["ycmp"] = np.ascontiguousarray(ycmp_all[i])
        in_maps.append(im)

    res = run_bass_kernel_spmd(nc, in_maps, core_ids=list(range(N_CORES)))
    global LAST_RESULT
    LAST_RESULT = res
    rew = np.concatenate([r["rew"].reshape(-1) for r in res.results])[:Q]
    return rew.astype(np.int32)


LAST_RESULT = None
